# revision 23
# baseline (speedup 1.0000x reference)
"""DenoiseNet loss kernel for 8 Trainium2 NeuronCores.

Data parallel over batch (4 batches/core). PointNet MLP in fp16 (fp32 PSUM
accumulate) with single-bank PSUM tiles on a 4-slot rotation so evacuation
copies pipeline under the next chunk's matmuls. Exact global BatchNorm via
per-layer AllReduce of (sum, sumsq): channel sums ride the matmul as a
501st rhs column (sum_z = W^T xsum, xsum accumulated by the previous
affine's accum_out), sumsq via one f16 square pass per half; affine+ReLU
applied in ONE Act pass (relu(a*z+c), per-channel AP scale/bias). KNN:
hi/lo-fp16 d' matmul, row-max, is_equal one-hot extraction; extraction of
iteration i is interleaved into iteration i+1's MLP (bursts at layer
boundaries) to fill AllReduce stalls. Loss partials summed on host.
"""
import numpy as np

import concourse.bass as bass
import concourse.mybir as mybir
import concourse.tile as tile
from concourse import bacc
from concourse.bass_utils import run_bass_kernel_spmd

dt = mybir.dt
F32 = dt.float32
F16 = dt.float16
F8 = dt.float8e4
AF = mybir.ActivationFunctionType
OP = mybir.AluOpType
AX = mybir.AxisListType
PM = mybir.MatmulPerfMode

B, N, NCORES = 32, 1000, 8
BL = B // NCORES            # 4 batches per core
PTS = BL * N                # 4000 points per core
NITER = 4
NG = B * N                  # 32000 (global BN population)
EPS = 1e-5
NOISE_DECAY = 4.0
QT = 125                    # query tile (8 per batch)
RP = 1024                   # padded ref points
CH = 500                    # column chunk (8 per core)

# (cin, cout, kind)
LAYERS = [(3, 64, 'f16'), (64, 128, 'f16'), (128, 256, 'f16'),
          (256, 512, 'f16'), (512, 1024, 'f16'), (1024, 512, 'f16'),
          (512, 256, 'f16'), (256, 3, 'out')]
NCO = [max(1, co // 128) for _, co, _ in LAYERS]
# raw f16 slot assignment per BN layer (12 slots, no layer's outputs
# overlap its own inputs)
RAWS = [[0], [1], [2, 3], [4, 5, 6, 7], [8, 9, 10, 11, 0, 1, 2, 3],
        [4, 5, 6, 7], [8, 9]]

RG = [list(range(NCORES))]

_NC_CACHE = {}


def fl3(t):
    """Flatten the two free dims of a [P, a, b] tile/AP."""
    ap = t[:] if hasattr(t, 'tile_pool_tag') or not hasattr(t, 'rearrange') else t
    try:
        return ap.rearrange("p a b -> p (a b)")
    except Exception:
        return t[:].rearrange("p a b -> p (a b)")


def _build(niter=NITER, nlayers=8, do_knn=True, do_ar=True, reps=1):
    nc = bacc.Bacc(None, target_bir_lowering=False, debug=False)

    x0t_d = nc.dram_tensor("x0t", [3, PTS], F32, kind="ExternalInput")
    cneg_d = nc.dram_tensor("cneg", [2, PTS], F16, kind="ExternalInput")
    cpos_d = nc.dram_tensor("cpos", [2, PTS], F16, kind="ExternalInput")
    sw_d = nc.dram_tensor("sw", [128, 32], F32, kind="ExternalInput")
    sws3_d = nc.dram_tensor("sws3", [3, PTS], F16, kind="ExternalInput")
    db3_d = nc.dram_tensor("db3t", [3, NITER], F32, kind="ExternalInput")
    r_d = [[nc.dram_tensor(f"rknn_{i}_{b}", [13, RP], F16, kind="ExternalInput")
            for b in range(BL)] for i in range(NITER)]
    w_d = []
    for i in range(NITER):
        row = []
        for l in range(8):
            cin, cout, kind = LAYERS[l]
            if cin > 128:
                shp, dty = [128, cin // 128, cout], F16
            else:
                shp, dty = [cin, cout], F16
            row.append(nc.dram_tensor(f"w_{i}_{l}", shp, dty,
                                      kind="ExternalInput"))
        w_d.append(row)
    gs_d = [[nc.dram_tensor(f"gs_{i}_{l}", [128, 16], F32,
                            kind="ExternalInput") for l in range(7)]
            for i in range(NITER)]
    loss_d = nc.dram_tensor("loss_part", [128, 1], F32, kind="ExternalOutput")
    loss3_d = nc.dram_tensor("loss_part3", [3, 1], F32, kind="ExternalOutput")

    inv_n = 1.0 / NG if do_ar else 1.0 / PTS

    with tile.TileContext(nc) as tc:
        with (
            tc.tile_pool(name="sb", bufs=1) as sb,
            tc.tile_pool(name="ps", bufs=2, space="PSUM") as ps,
            tc.tile_pool(name="psk", bufs=2, space="PSUM") as psk,
            tc.tile_pool(name="dram", bufs=2, space="DRAM") as dram,
        ):
          for rep in range(reps):
            # ---------------- persistent setup ----------------
            sw_sb = sb.tile([128, 32], F32, tag="sw")
            nc.gpsimd.dma_start(sw_sb[:], sw_d[:])
            sws3 = sb.tile([3, 8, CH], F16, tag="sws3")
            nc.gpsimd.dma_start(fl3(sws3), sws3_d[:])
            db3_sb = sb.tile([3, NITER], F32, tag="db3")
            nc.gpsimd.dma_start(db3_sb[:], db3_d[:])

            x_cur = sb.tile([3, 8, CH], F32, tag="xA")
            nc.gpsimd.dma_start(fl3(x_cur), x0t_d[:])

            Ld = sb.tile([11, PTS], F16, tag="Ld")
            nc.gpsimd.dma_start(Ld[9:11, :], cneg_d[:])
            Le = sb.tile([11, PTS], F16, tag="Le")
            nc.gpsimd.dma_start(Le[9:11, :], cpos_d[:])

            eps_sb = sb.tile([128, 1], F32, tag="epsc")
            nc.vector.memset(eps_sb[:], float(EPS))
            licol = sb.tile([128, NITER], F32, tag="licol")
            nc.vector.memset(licol[:], 0.0)
            a3col = sb.tile([3, NITER], F32, tag="a3col")
            nc.vector.memset(a3col[:], 0.0)
            S = sb.tile([128, 32], F32, tag="S")
            nc.vector.memset(S[:], 0.0)
            m_all = sb.tile([128, 32], F32, tag="m_all")

            # activation storage
            raw = [sb.tile([128, 8, CH + 1], F16, tag=f"rw{s}", name=f"raw{s}")
                   for s in range(12)]
            sc16 = sb.tile([3, 8, CH + 1], F16, tag="sc16")
            sqjk = sb.tile([128, 8, CH], F16, tag="sqjk")

            # extraction queue state: (it, r_tiles, Ld_t, Le_t) groups
            ext_q = []
            ext_it = [None]  # iter whose groups are in ext_q

            def emit_ext(nmax):
                for _ in range(min(nmax, len(ext_q))):
                    b, qt_i, r_t, Ld_t, Le_t, it_src = ext_q.pop(0)
                    col = b * 8 + qt_i
                    qsl = slice(b * N + qt_i * QT, b * N + (qt_i + 1) * QT)
                    kpd = psk.tile([QT, 2, 512], F32, tag="kp")
                    for rt in range(2):
                        nc.tensor.matmul(kpd[:, rt, :], Ld_t[0:11, qsl],
                                         r_t[b][0:11, rt * 512:(rt + 1) * 512],
                                         start=True, stop=True)
                    nc.vector.tensor_reduce(m_all[0:QT, col:col + 1],
                                            kpd[:], AX.XY, OP.max)
                    kpe = psk.tile([QT, 2, 512], F32, tag="kp")
                    for rt in range(2):
                        nc.tensor.matmul(kpe[:, rt, :], Le_t[0:11, qsl],
                                         r_t[b][0:11, rt * 512:(rt + 1) * 512],
                                         start=True, stop=True)
                    e16 = sb.tile([QT, 2, 512], F16, tag="e16", bufs=2)
                    nc.scalar.activation(e16[:], kpe[:], AF.Copy)
                    jk = sb.tile([QT, 2, 512], F16, tag="jk", bufs=2)
                    nc.vector.scalar_tensor_tensor(
                        jk[:], kpd[:], m_all[0:QT, col:col + 1], e16[:],
                        OP.is_equal, OP.mult,
                        accum_out=S[0:QT, col:col + 1])
                    if not ext_q:  # queue drained: fold S into licol
                        it_s = ext_it[0]
                        jk2 = sb.tile([128, 32], F16, tag="jk2", bufs=2)
                        nc.vector.scalar_tensor_tensor(
                            jk2[:], S[:], 1.0, sw_sb[:], OP.mult, OP.mult,
                            accum_out=licol[:, it_s:it_s + 1])
                        nc.vector.memset(S[:], 0.0)

            for it in range(niter):
                # r tiles for this iteration (parity tags: extraction of
                # it-1 still reads the other parity during this iter)
                r_sb = []
                for b in range(BL):
                    rt_ = sb.tile([13, RP], F16, tag=f"r{it % 2}_{b}")
                    nc.gpsimd.dma_start(rt_[:], r_d[it][b][:])
                    r_sb.append(rt_)

                # weights + gs for this iter
                wt_d, gs_t = [], []
                for l in range(nlayers):
                    cin, cout, kind = LAYERS[l]
                    if cin > 128:
                        w = sb.tile([128, cin // 128, cout], F16,
                                    tag=f"w16_{l}", bufs=1, name=f"wt{l}")
                    else:
                        w = sb.tile([cin, cout], F16, tag=f"w16_{l}",
                                    bufs=1, name=f"wt{l}")
                    nc.gpsimd.dma_start(w[:], w_d[it][l][:])
                    wt_d.append(w)
                    if l < 7:
                        g = sb.tile([128, 16], F32, tag=f"gs_{l}", bufs=1)
                        nc.gpsimd.dma_start(g[:], gs_d[it][l][:])
                        gs_t.append(g)

                # fp16 input copy for L1 (+ channel sum into col 500)
                with nc.allow_low_precision("xsum col is f16; mean err ~1e-4"):
                    nc.vector.tensor_scalar(sc16[0:3, :, 0:CH], x_cur[:], 0.0,
                                            0.0, OP.add, OP.add,
                                            accum_out=sc16[0:3, 0, CH:CH + 1])

                rhs_raw = None      # raw-slot list of previous f16 layer
                for l in range(nlayers):
                    cin, cout, kind = LAYERS[l]
                    nco = NCO[l]
                    is_out = kind == 'out'
                    nchunk = max(1, cin // 128)
                    CIP = min(128, cin)
                    if not is_out:
                        slots = RAWS[l]
                        statsr = sb.tile([128, 24], F32, tag="statsr", bufs=2)
                        nc.vector.memset(statsr[:], 0.0)

                    for co in range(nco):
                        CO = min(128, cout - co * 128)
                        for h in range(2):
                            zps = [ps.tile([128, 512], F32, tag="zp",
                                            bufs=4,
                                            name=f"zp_{l}_{co}_{h}_{jj}")
                                   for jj in range(4)]
                            for ci in range(nchunk):
                                if cin > 128:
                                    lhsT = wt_d[l][:, ci,
                                                   co * 128:co * 128 + CO]
                                else:
                                    lhsT = wt_d[l][0:CIP,
                                                   co * 128:co * 128 + CO]
                                for j in range(4):
                                    c = h * 4 + j
                                    w = CH + 1 if (c == 0 and
                                                   not is_out) else CH
                                    src_t = sc16 if l == 0 else rhs_raw[ci]
                                    rhs = src_t[0:CIP, c, 0:w]
                                    nc.tensor.matmul(
                                        zps[j][0:CO, 0:w],
                                        lhsT, rhs,
                                        start=(ci == 0),
                                        stop=(ci == nchunk - 1))
                            # harvest the channel-sum column (h=0 only)
                            if h == 0 and not is_out:
                                nc.vector.tensor_copy(
                                    statsr[0:CO, co:co + 1],
                                    zps[0][0:CO, 500:501])
                            # evacuate PSUM (one copy per bank)
                            for j in range(4):
                                c = h * 4 + j
                                if is_out:
                                    nc.scalar.activation(
                                        sc16[0:3, c, 0:CH],
                                        zps[j][0:3, 0:CH], AF.Tanh,
                                        bias=db3_sb[:, it:it + 1])
                                    continue
                                dst = raw[slots[co]][0:CO, c, 0:CH]
                                if j == 1:
                                    nc.vector.tensor_copy(
                                        dst, zps[j][0:CO, 0:CH])
                                else:
                                    nc.scalar.activation(
                                        dst, zps[j][0:CO, 0:CH], AF.Copy)
                            if not is_out:
                                # sumsq of this half (shortens stats tail)
                                nc.vector.scalar_tensor_tensor(
                                    sqjk[0:CO, 4 * h:4 * h + 4, :],
                                    raw[slots[co]][0:CO, 4 * h:4 * h + 4,
                                                   0:CH], 1.0,
                                    raw[slots[co]][0:CO, 4 * h:4 * h + 4,
                                                   0:CH],
                                    OP.mult, OP.mult,
                                    accum_out=statsr[0:CO, 8 + 8 * h + co:
                                                     9 + 8 * h + co])
                                if do_knn and l in (4, 5) and h == 0:
                                    emit_ext(1)
                        if do_knn:
                            emit_ext(1)

                    if is_out:
                        break

                    arin = dram.tile([128, 24], F32, tag="arin")
                    arout = dram.tile([128, 24], F32, tag="arout")
                    nc.gpsimd.dma_start(arin[:], statsr[:])
                    if do_ar:
                        nc.gpsimd.collective_compute(
                            "AllReduce", OP.add, replica_groups=RG,
                            ins=[arin.opt()], outs=[arout.opt()])
                    else:
                        nc.gpsimd.dma_start(arout[:], arin[:])
                    statsg = sb.tile([128, 24], F32, tag="statsg", bufs=2)
                    nc.gpsimd.dma_start(statsg[:], arout[:])

                    # affine coefficients: a = g*rsqrt(var+eps), c = b - m*a
                    af = sb.tile([128, 48], F32, tag="af", bufs=2)
                    gl = gs_t[l]
                    sl = slice(0, nco)

                    def L(i, s=sl):
                        return af[:, 8 * i + s.start: 8 * i + s.stop]

                    nc.vector.tensor_scalar_mul(L(0), statsg[:, 0:nco], inv_n)
                    nc.vector.tensor_tensor(out=L(1), in0=statsg[:, 8:8 + nco],
                                            in1=statsg[:, 16:16 + nco],
                                            op=OP.add)
                    nc.vector.tensor_scalar_mul(L(1), L(1), inv_n)
                    nc.vector.tensor_tensor(out=L(2), in0=L(0), in1=L(0),
                                            op=OP.mult)
                    nc.vector.tensor_tensor(out=L(2), in0=L(1), in1=L(2),
                                            op=OP.subtract)
                    nc.scalar.activation(L(3), L(2), AF.Sqrt, bias=eps_sb[:])
                    nc.vector.reciprocal(L(1), L(3))
                    nc.vector.tensor_tensor(out=L(4), in0=gl[:, 0:nco],
                                            in1=L(1), op=OP.mult)
                    nc.vector.tensor_tensor(out=L(2), in0=L(0), in1=L(4),
                                            op=OP.mult)
                    nc.vector.tensor_tensor(out=L(5), in0=gl[:, 8:8 + nco],
                                            in1=L(2), op=OP.subtract)

                    # affine+relu, one Act pass per co (+ xsum for next BN)
                    for co in range(nco):
                        CO = min(128, cout - co * 128)
                        srcv = raw[slots[co]][0:CO, :, 0:CH]
                        with nc.allow_low_precision("xsum col f16"):
                            nc.scalar.activation(
                                srcv, srcv, AF.Relu,
                                bias=af[0:CO, 40 + co:41 + co],
                                scale=af[0:CO, 32 + co:33 + co],
                                accum_out=raw[slots[co]][0:CO, 0,
                                                         CH:CH + 1])
                    rhs_raw = [raw[s] for s in slots]
                    if do_knn:
                        emit_ext(2)

                if nlayers < 8:
                    continue

                # ---- build Ld from pre-update x (for THIS iter's KNN) ----
                if do_knn:
                    nc.vector.tensor_scalar_mul(Ld[0:3, :],
                                                fl3(x_cur), 2.0)
                    lo = fl3(sqjk[0:3, :, :])
                    nc.vector.scalar_tensor_tensor(lo, fl3(x_cur),
                                                   2.0, Ld[0:3, :], OP.mult,
                                                   OP.subtract)
                    nc.gpsimd.dma_start(Ld[3:6, :], lo)
                    nc.gpsimd.dma_start(Ld[6:9, :], Ld[0:3, :])

                # ---- x += pred ----
                nc.vector.tensor_tensor(out=x_cur[:], in0=x_cur[:],
                                        in1=sc16[0:3, :, 0:CH], op=OP.add)

                if not do_knn:
                    continue

                # ---- build Le from updated x ----
                nc.vector.tensor_scalar_mul(Le[0:3, :], fl3(x_cur),
                                            -2.0)
                lo2 = fl3(sqjk[0:3, :, :])
                nc.vector.scalar_tensor_tensor(lo2, fl3(x_cur),
                                               -2.0, Le[0:3, :], OP.mult,
                                               OP.subtract)
                nc.gpsimd.dma_start(Le[3:6, :], lo2)
                nc.gpsimd.dma_start(Le[6:9, :], Le[0:3, :])

                # ---- term2: sum_q sw_q |x_new_q|^2 ----
                wv = sc16[0:3, :, 0:CH]
                nc.vector.tensor_tensor(out=wv, in0=x_cur[:],
                                        in1=sws3[:], op=OP.mult)
                nc.vector.scalar_tensor_tensor(
                    sqjk[0:3, :, :], wv, 1.0, wv,
                    OP.mult, OP.mult, accum_out=a3col[0:3, it:it + 1])

                # queue this iteration's extraction groups
                assert not ext_q
                ext_it[0] = it
                for b in range(BL):
                    for qt_i in range(8):
                        ext_q.append((b, qt_i, r_sb, Ld, Le, it))
                if it == niter - 1:
                    emit_ext(len(ext_q))

            lacc = sb.tile([128, 1], F32, tag="laccA")
            nc.vector.tensor_reduce(lacc[:], licol[:], AX.X, OP.add)
            acc3f = sb.tile([3, 1], F32, tag="acc3A")
            nc.vector.tensor_reduce(acc3f[:], a3col[0:3, :], AX.X, OP.add)
            nc.gpsimd.dma_start(loss_d[:], lacc[:])
            nc.gpsimd.dma_start(loss3_d[:], acc3f[:])
    nc.compile()
    return nc


def _host_prep(inputs):
    """Build per-core input maps."""
    f32 = np.float32
    noisy = np.asarray(inputs["pcl_noisy"], f32)
    clean = np.asarray(inputs["pcl_clean"], f32)
    seeds = np.asarray(inputs["pcl_seeds"], f32)
    std = np.asarray(inputs["pcl_std"], f32)
    noise = np.asarray(inputs["noise"], f32)

    pn = noisy - seeds
    pc = clean - seeds
    sdist = np.sum(pn.astype(np.float64) ** 2, -1, keepdims=True)
    max_sq = sdist[:, -1:, :]
    sw = np.exp(-sdist * 9.0 / max_sq)[..., 0]
    sw = (sw / sw.sum(1, keepdims=True))  # [B, N] float64

    tgts = []
    cur = std.copy()
    for i in range(NITER):
        if i < NITER - 1:
            cur = cur / NOISE_DECAY
            tgts.append(pc + noise[i] * cur[:, None, None])
        else:
            tgts.append(pc.copy())

    sent = np.full((RP - N, 3), 100.0, np.float64)

    shared = {}
    for i in range(NITER):
        for l in range(8):
            cin, cout, kind = LAYERS[l]
            key = f'ew{l+1}' if l < 5 else f'dw{l-4}'
            W = np.asarray(inputs[key], f32)[i]
            if cin > 128:
                arr = W.reshape(cin // 128, 128, cout).transpose(1, 0, 2)
                shared[f"w_{i}_{l}"] = np.ascontiguousarray(arr).astype(
                    np.float16)
            else:
                shared[f"w_{i}_{l}"] = W.astype(np.float16)
        for l in range(7):
            nco = NCO[l]
            cout = LAYERS[l][1]
            gk = f'eg{l+1}' if l < 5 else f'dg{l-4}'
            hk = f'eh{l+1}' if l < 5 else f'dh{l-4}'
            g = np.asarray(inputs[gk], f32)[i]
            h = np.asarray(inputs[hk], f32)[i]
            arr = np.zeros((128, 16), f32)
            gp = np.zeros(nco * 128, f32); gp[:cout] = g
            hp = np.zeros(nco * 128, f32); hp[:cout] = h
            arr[:, 0:nco] = gp.reshape(nco, 128).T
            arr[:, 8:8 + nco] = hp.reshape(nco, 128).T
            shared[f"gs_{i}_{l}"] = arr
    shared["db3t"] = np.ascontiguousarray(np.asarray(inputs["db3"], f32).T)
    shared["cneg"] = np.full((2, PTS), -1.0, np.float16)
    shared["cpos"] = np.full((2, PTS), 1.0, np.float16)

    in_maps = []
    for c in range(NCORES):
        bs = slice(c * BL, (c + 1) * BL)
        m = dict(shared)
        m["x0t"] = np.ascontiguousarray(
            pn[bs].transpose(2, 0, 1).reshape(3, PTS))
        swc = np.zeros((128, 32), f32)
        for b in range(BL):
            for qt in range(8):
                swc[0:QT, b * 8 + qt] = sw[c * BL + b,
                                           qt * QT:(qt + 1) * QT].astype(f32)
        m["sw"] = swc
        m["sws3"] = np.broadcast_to(
            np.sqrt(sw[bs]).reshape(1, PTS), (3, PTS)).astype(np.float16)
        for i in range(NITER):
            for b in range(BL):
                coords = np.concatenate(
                    [tgts[i][c * BL + b].astype(np.float64), sent], 0)
                rh = coords.astype(np.float16)
                rl = (coords - rh.astype(np.float64)).astype(np.float16)
                rsq = (coords ** 2).sum(1)
                rsqh = rsq.astype(np.float16)
                rsql = (rsq - rsqh.astype(np.float64)).astype(np.float16)
                R = np.empty((13, RP), np.float16)
                R[0:3] = rh.T
                R[3:6] = rh.T
                R[6:9] = rl.T
                R[9] = rsqh
                R[10] = rsql
                R[11] = 1.0
                R[12] = 1.0
                m[f"rknn_{i}_{b}"] = R
        in_maps.append(m)
    return in_maps


def kernel(**inputs):
    if "nc" not in _NC_CACHE:
        _NC_CACHE["nc"] = _build()
    nc = _NC_CACHE["nc"]
    in_maps = _host_prep(inputs)
    res = run_bass_kernel_spmd(nc, in_maps, list(range(NCORES))).results
    total = 0.0
    for c in range(NCORES):
        total += float(res[c]["loss_part"].sum())
        total += float(res[c]["loss_part3"].sum())
    return np.asarray(total / B, dtype=np.float32)
